# revision 24
# baseline (speedup 1.0000x reference)
"""MoE (top-k of 8 experts) Trainium2 kernel — mixed fp16/fp8 tiers.

Strategy (expert parallelism + per-assignment precision tiering):
  - Host computes gating (float64 softmax/top-k/renorm) exactly as the
    reference.
  - Each (token, expert) assignment runs either the fp16 path (512 PE
    cyc/token) or, when its combine weight c is small, the full-fp8
    path (256 cyc/token): fp8 e4m3 matmuls in DoubleRow perf mode
    process K=256 per instruction at the fp16 row rate (2x FLOPs).
    Measured e4m3 pipeline error ~7.7% x c per fp8 assignment; tokens
    are tiered so total rel err stays ~1.6e-2 (< 2e-2 gate).
  - Weights pre-scaled before fp8 quantization (W1 x32, W2 x64) to
    escape e4m3's subnormal range; dequant is folded into the ACT
    scale (GEMM1) and the host-packed per-token combine weights
    (GEMM2).
  - Packing: per-core 2 mega-slots (SA, SB) as in the fp16 baseline,
    each mega = one expert's tokens with a leading tier3 (fp8) block
    region (a3 / b3 tokens) and a fp16 tail; per-expert tier3 counts
    are capacity-driven with an error-threshold feasibility check.
  - Host scatter-adds expert contributions + combine-weighted b2.

Device kernel (per core, per mega, per 1024-row weight block hb):
  GEMM1 fp8 : psum[h,t] = sum_k2 (32*W1)_8[.,k2,2,h].T @ x8[.,k2,2,t]
              (DoubleRow), ACT: ht8 = fp8(gelu(psum/32 + b1)).
  GEMM1 fp16: baseline path -> ht16 = fp16(gelu(psum + b1)).
  GEMM2 fp8 : ps2[t,d] += ht8[.,k2,2,t].T @ (64*W2)_8[.,k2,2,d]
              (DoubleRow, 4 k-steps), DVE-accumulated into yas.
  GEMM2 fp16: baseline path (8 k-steps).
  yas scaled by host wt' (wt/64 for fp8 blocks) and stored per ts.
"""

import os
import numpy as np
import ml_dtypes

D = 1024
H = 4096
E = 8
N_CORES = 8
HBLK = 1024          # h rows per streamed weight block
HB = H // HBLK       # 4 blocks
KD = D // 128        # 8 k128-tiles for GEMM1 (fp16)
KD2 = KD // 2        # 4 k256-tiles for GEMM1 (fp8 DoubleRow)
KHB = HBLK // 128    # 8 k128-tiles per block for GEMM2 (fp16)
KHB2 = KHB // 2      # 4 k256-tiles for GEMM2 (fp8)

W1_SCALE = 32.0
W2_SCALE = 64.0
TH_FLOOR = 0.37      # fp8 if c <= floor (when capacity allows)
# Predicted rel err of fp8 tiering ~= ERR_K * sqrt(sum of c^2 over fp8
# assignments); ERR_K calibrated by exact host sim of the e4m3 pipeline
# on reference-scale inputs.  Budget keeps predicted rel <= ~1.7e-2.
ERR_K = 8.09e-4
SUMC2_BUDGET = 452.0

NPF8 = ml_dtypes.float8_e4m3


def _slice_period(n):
    # fp16 matmul issue period (measured): N/2.4GHz + ~3ns dispatch,
    # with a ~100ns floor where the FWL LDWEIGHTS (~97ns) stops being
    # hidden by the moving-operand stream.
    return max(n / 2.4 + 3.0, 100.0)


def _best_slices(mega):
    """DP: split mega into moving-dim slices (multiples of 64, <=512)
    minimizing the summed matmul issue period."""
    if mega == 0:
        return []
    best = {0: (0.0, ())}
    for m in range(64, mega + 64, 64):
        cands = []
        for s in range(64, min(512, m) + 64, 64):
            if m - s in best:
                c, parts = best[m - s]
                cands.append((c + _slice_period(s), parts + (s,)))
        if cands:
            best[m] = min(cands)
    assert mega in best, f"no slice decomposition for {mega}"
    _, parts = best[mega]
    out = []
    off = 0
    for s in parts:
        out.append((off, s))
        off += s
    return out


_KERNEL_CACHE = {}
LAST_EXEC_NS = None


def _build_kernel(megas):
    """megas: tuple of (size, n3) per mega; n3 = leading fp8 tokens
    (multiple of 128), size multiple of 128."""
    import concourse.bacc as bacc
    import concourse.mybir as mybir
    import concourse.tile as tile

    f32 = mybir.dt.float32
    f16 = mybir.dt.float16
    f8 = mybir.dt.float8e4
    GELU = mybir.ActivationFunctionType.Gelu_apprx_tanh
    DR = mybir.MatmulPerfMode.DoubleRow

    C = sum(sz for sz, _ in megas)
    C3 = sum(n3 for _, n3 in megas)
    nc = bacc.Bacc("TRN2", target_bir_lowering=False, debug=False,
                   num_devices=N_CORES)

    # host-swizzled layouts matching SBUF tile layouts (128-row DMAs of
    # long contiguous runs):
    #   xT16[p, per-slice (kk, c)]   xT8[p, per-slice (k2, 2, c)]
    #   w1f16[p, hb, kk, hw]         w1f8[p, hb, k2, hs, 2, 128]
    #   w2f16[p, hb, kh, d]          w2f8[p, hb, k2, 2, d]
    xT16 = (nc.dram_tensor("xT16", [128, (C - C3) * KD], f16,
                           kind="ExternalInput").ap()
            if C > C3 else None)
    xT8 = (nc.dram_tensor("xT8", [128, C3 * KD], f8,
                          kind="ExternalInput").ap()
           if C3 else None)
    wts = []
    for mi, (sz, n3) in enumerate(megas):
        ww = {}
        if sz > n3:
            ww["w1f16"] = nc.dram_tensor(f"w1f16_{mi}", [128, HB, KD, HBLK],
                                         f16, kind="ExternalInput").ap()
            ww["w2f16"] = nc.dram_tensor(f"w2f16_{mi}", [128, HB, KHB, D],
                                         f16, kind="ExternalInput").ap()
        if n3:
            ww["w1f8"] = nc.dram_tensor(f"w1f8_{mi}",
                                        [128, HB, KD2, KHB, 2, 128],
                                        f8, kind="ExternalInput").ap()
            ww["w2f8"] = nc.dram_tensor(f"w2f8_{mi}", [128, HB, KD2, 2, D],
                                        f8, kind="ExternalInput").ap()
        # pre-transposed on host: [128, H/128], col j = b1[j*128 + p]
        ww["b1"] = nc.dram_tensor(f"b1_{mi}", [128, H // 128], f32,
                                  kind="ExternalInput").ap()
        wts.append(ww)
    # pre-transposed + tier-scaled on host: [128, C/128]
    wt = nc.dram_tensor("wt", [128, C // 128], f32,
                        kind="ExternalInput").ap()
    # mega0 W1 block-0 prestage: fp8 part as two fast-issue chunks (the
    # first fp8 matmuls wait only on the small "a" chunk); if mega0 has
    # no fp8 region, prestage the fp16 block instead.
    m0_has8 = megas[0][1] > 0
    if m0_has8:
        w1h0a = nc.dram_tensor("w1h0a", [128, KD2 * 2 * 2 * 128], f8,
                               kind="ExternalInput").ap()
        w1h0b = nc.dram_tensor("w1h0b", [128, KD2 * 6 * 2 * 128], f8,
                               kind="ExternalInput").ap()
    else:
        w1h0a = nc.dram_tensor("w1h0a", [128, KD * 256], f16,
                               kind="ExternalInput").ap()
        w1h0b = nc.dram_tensor("w1h0b", [128, KD * 768], f16,
                               kind="ExternalInput").ap()
    y = nc.dram_tensor("y", [C, D], f32, kind="ExternalOutput").ap()

    with tile.TileContext(nc) as tc:
        with (
            tc.tile_pool(name="meta", bufs=1) as pmeta,
            tc.tile_pool(name="xg", bufs=3) as pxg,
            tc.tile_pool(name="yacc", bufs=10) as pyacc,
            tc.tile_pool(name="w1p", bufs=2) as pw1,
            tc.tile_pool(name="w1p8", bufs=2) as pw18,
            tc.tile_pool(name="w2p", bufs=1) as pw2,
            tc.tile_pool(name="w2p8", bufs=1) as pw28,
            tc.tile_pool(name="hact", bufs=1) as phact,
            tc.tile_pool(name="ps1", bufs=4, space="PSUM") as pps1,
            tc.tile_pool(name="ps2", bufs=4, space="PSUM") as pps2,
        ):
            y_r = y.rearrange("(t p) d -> p t d", p=128)
            wtt = None

            # PE warmup on zeros during the DMA head (pstate ramp).
            warm = pmeta.tile([128, 512], f16, name="warm")
            nc.vector.memset(warm[:], 0.0)
            # preload the gelu ACT table (lazy-loads on first ACTIVATE,
            # ~1.3us) off the critical path
            warmact = pmeta.tile([128, 1], f32, name="warmact")
            nc.scalar.activation(warmact[:], warm[:, :1], GELU)
            for wi in range(14):
                pw = pps1.tile([128, 512], f32, tag="ps1",
                               name=f"warm_ps_{wi}")
                nc.tensor.matmul(pw[:], warm[:, :128], warm[:],
                                 start=True, stop=True)

            if m0_has8:
                w1h0a_t = pmeta.tile([128, KD2, 2, 2, 128], f8,
                                     name="w1h0a_t")
                nc.sync.dma_start(
                    w1h0a_t[:],
                    w1h0a.rearrange("p (k s i m) -> p k s i m",
                                    k=KD2, s=2, i=2))
                w1h0b_t = pmeta.tile([128, KD2, 6, 2, 128], f8,
                                     name="w1h0b_t")
                nc.sync.dma_start(
                    w1h0b_t[:],
                    w1h0b.rearrange("p (k s i m) -> p k s i m",
                                    k=KD2, s=6, i=2))
            else:
                w1h0a_t = pmeta.tile([128, KD, 256], f16, name="w1h0a_t")
                nc.sync.dma_start(
                    w1h0a_t[:], w1h0a.rearrange("p (kk h) -> p kk h", kk=KD))
                w1h0b_t = pmeta.tile([128, KD, 768], f16, name="w1h0b_t")
                nc.sync.dma_start(
                    w1h0b_t[:], w1h0b.rearrange("p (kk h) -> p kk h", kk=KD))

            off = 0        # token offset into C
            off16 = 0      # fp16 token offset (into xT16)
            off8 = 0       # fp8 token offset (into xT8)
            for mi, (mega, n3) in enumerate(megas):
                ww = wts[mi]
                n1 = mega - n3
                ts_count = mega // 128
                nt3 = n3 // 128          # fp8 ts-blocks (leading)
                ts0 = off // 128
                sl3 = _best_slices(n3)
                sl1 = _best_slices(n1)

                # per-slice token loads (gpsimd queue so they don't
                # serialize behind the weight streams)
                xg8s = []
                for (soff, slen) in sl3:
                    xg = pxg.tile([128, KD2, 2, slen], f8, tag="xg8",
                                  name=f"xg8_{mi}_{soff}")
                    base = (off8 + soff) * KD
                    nc.gpsimd.dma_start(
                        xg[:],
                        xT8[:, base:base + slen * KD]
                        .rearrange("p (k i c) -> p k i c", k=KD2, i=2))
                    xg8s.append(xg)
                xg16s = []
                for (soff, slen) in sl1:
                    xg = pxg.tile([128, KD, slen], f16, tag="xg16",
                                  name=f"xg16_{mi}_{soff}")
                    base = (off16 + soff) * KD
                    # scalar queue: keeps the head's gpsimd/sync queues
                    # free for the critical fp8 prestage + x8 loads
                    nc.scalar.dma_start(
                        xg[:],
                        xT16[:, base:base + slen * KD]
                        .rearrange("p (kk c) -> p kk c", kk=KD))
                    xg16s.append(xg)

                b1t = pmeta.tile([128, H // 128], f32, tag=f"b1_{mi}")
                nc.sync.dma_start(b1t[:], ww["b1"][:])
                if wtt is None:
                    wtt = pmeta.tile([128, C // 128], f32, name="wtt")
                    nc.sync.dma_start(wtt[:], wt[:])

                yas = [pyacc.tile([128, D], f32, tag="ya",
                                  name=f"ya_{mi}_{ts}")
                       for ts in range(ts_count)]

                for hb in range(HB):
                    first_blk = mi == 0 and hb == 0
                    # --- weight block loads (sync queue); issue order =
                    # consumption order after the per-hb reordering:
                    # w1f8, w2f8, w1f16, w2f16 ---
                    if n3:
                        if first_blk and m0_has8:
                            w1t8 = None     # served from w1h0a/b tiles
                        else:
                            w1t8 = pw18.tile([128, KD2, KHB, 2, 128], f8,
                                             tag="w1t8")
                            nc.sync.dma_start(w1t8[:], ww["w1f8"][:, hb])
                        w2t8 = pw28.tile([128, KD2, 2, D], f8, tag="w2t8")
                        nc.sync.dma_start(w2t8[:], ww["w2f8"][:, hb])
                    if n1:
                        if first_blk and not m0_has8:
                            w1t = None      # served from w1h0a/b tiles
                        else:
                            w1t = pw1.tile([128, KD, HBLK], f16, tag="w1t")
                            nc.sync.dma_start(w1t[:], ww["w1f16"][:, hb])
                        w2t = pw2.tile([128, KHB, D], f16, tag="w2t")
                        nc.sync.dma_start(w2t[:], ww["w2f16"][:, hb])

                    if n3:
                        ht8 = phact.tile([128, KHB2, 2, n3], f8, tag="ht8")
                    if n1:
                        ht = phact.tile([128, KHB, n1], f16, tag="ht")

                    # A small fp8 slice (< ~320 rows) is LDWEIGHTS-bound
                    # (135ns DR weight load > moving-row time); in that
                    # case interleave its matmuls into the first fp16
                    # slice's stream so the LDW engine stays ahead.
                    ilv = (n3 > 0 and n1 > 0 and len(sl3) == 1
                           and sl3[0][1] <= 320 and not first_blk)

                    # --- GEMM1 fp8 (DoubleRow) + gelu -> ht8 ---
                    if not ilv:
                        for si, (soff, slen) in enumerate(sl3):
                            for hs in range(KHB):
                                ps = pps1.tile([128, 512], f32, tag="ps1")
                                for k in range(KD2):
                                    if first_blk and m0_has8:
                                        w1s = (w1h0a_t[:, k, hs, :, :]
                                               if hs < 2 else
                                               w1h0b_t[:, k, hs - 2, :, :])
                                    else:
                                        w1s = w1t8[:, k, hs, :, :]
                                    nc.tensor.matmul(
                                        ps[:, :slen],
                                        w1s,
                                        xg8s[si][:, k, :, :],
                                        start=(k == 0), stop=(k == KD2 - 1),
                                        perf_mode=DR,
                                    )
                                nc.scalar.activation(
                                    ht8[:, hs // 2, hs % 2, soff:soff + slen],
                                    ps[:, :slen], GELU,
                                    bias=b1t[:, hb * KHB + hs:hb * KHB + hs + 1],
                                    scale=1.0 / W1_SCALE,
                                )
                    else:
                        slen3 = sl3[0][1]
                        s0off, s0len = sl1[0]
                        for hs in range(KHB):
                            bias = b1t[:, hb * KHB + hs:hb * KHB + hs + 1]
                            ps8 = pps1.tile([128, 512], f32, tag="ps1")
                            ps16 = pps1.tile([128, 512], f32, tag="ps1")
                            k8 = 0
                            for k in range(KD):
                                nc.tensor.matmul(
                                    ps16[:, :s0len],
                                    w1t[:, k, hs * 128:(hs + 1) * 128],
                                    xg16s[0][:, k, :],
                                    start=(k == 0), stop=(k == KD - 1),
                                    skip_group_check=True,
                                )
                                if k % 2 == 1:
                                    nc.tensor.matmul(
                                        ps8[:, :slen3],
                                        w1t8[:, k8, hs, :, :],
                                        xg8s[0][:, k8, :, :],
                                        start=(k8 == 0),
                                        stop=(k8 == KD2 - 1),
                                        perf_mode=DR,
                                        skip_group_check=True,
                                    )
                                    k8 += 1
                            nc.scalar.activation(
                                ht8[:, hs // 2, hs % 2, :slen3],
                                ps8[:, :slen3], GELU, bias=bias,
                                scale=1.0 / W1_SCALE,
                            )
                            nc.scalar.activation(
                                ht[:, hs, s0off:s0off + s0len],
                                ps16[:, :s0len], GELU, bias=bias,
                            )

                    # --- GEMM2 fp8 (before the fp16 sections: hides the
                    # fp16 W1 stream latency behind fp8 work) ---
                    for ts in range(nt3):
                        for dh in range(2):
                            ps2 = pps2.tile([128, 512], f32, tag="ps2")
                            for k in range(KHB2):
                                nc.tensor.matmul(
                                    ps2[:],
                                    ht8[:, k, :, ts * 128:(ts + 1) * 128],
                                    w2t8[:, k, :, dh * 512:(dh + 1) * 512],
                                    start=(k == 0), stop=(k == KHB2 - 1),
                                    perf_mode=DR,
                                )
                            dst = yas[ts][:, dh * 512:(dh + 1) * 512]
                            if hb == 0:
                                nc.vector.tensor_copy(dst, ps2[:])
                            else:
                                nc.vector.tensor_add(dst, dst, ps2[:])
                        if hb == HB - 1:
                            nc.vector.tensor_scalar_mul(
                                yas[ts][:], yas[ts][:],
                                wtt[:, ts0 + ts:ts0 + ts + 1])
                            q = nc.gpsimd if ts % 2 == 0 else nc.sync
                            q.dma_start(y_r[:, ts0 + ts, :], yas[ts][:])

                    # --- GEMM1 fp16 + gelu -> ht ---
                    if first_blk and not m0_has8:
                        ns = len(sl1)
                        order = ([(si, hs) for si in range(min(2, ns))
                                  for hs in range(2)]
                                 + [(si, hs) for si in range(min(2, ns))
                                    for hs in range(2, KHB)]
                                 + [(si, hs) for si in range(2, ns)
                                    for hs in range(KHB)])
                    else:
                        # slice 0 already done in the interleaved loop
                        si0 = 1 if ilv else 0
                        order = [(si, hs) for si in range(si0, len(sl1))
                                 for hs in range(KHB)]
                    for si, hs in order:
                        soff, slen = sl1[si]
                        ps = pps1.tile([128, 512], f32, tag="ps1")
                        for k in range(KD):
                            if first_blk and not m0_has8:
                                w1s = (
                                    w1h0a_t[:, k, hs * 128:(hs + 1) * 128]
                                    if hs < 2 else
                                    w1h0b_t[:, k,
                                            (hs - 2) * 128:(hs - 1) * 128])
                            else:
                                w1s = w1t[:, k, hs * 128:(hs + 1) * 128]
                            nc.tensor.matmul(
                                ps[:, :slen],
                                w1s,
                                xg16s[si][:, k, :],
                                start=(k == 0), stop=(k == KD - 1),
                            )
                        nc.scalar.activation(
                            ht[:, hs, soff:soff + slen], ps[:, :slen],
                            GELU,
                            bias=b1t[:, hb * KHB + hs:hb * KHB + hs + 1],
                        )

                    # --- GEMM2 fp16 ---
                    for ts in range(nt3, ts_count):
                        tl = ts * 128 - n3
                        for dh in range(2):
                            ps2 = pps2.tile([128, 512], f32, tag="ps2")
                            for k in range(KHB):
                                nc.tensor.matmul(
                                    ps2[:],
                                    ht[:, k, tl:tl + 128],
                                    w2t[:, k, dh * 512:(dh + 1) * 512],
                                    start=(k == 0), stop=(k == KHB - 1),
                                )
                            dst = yas[ts][:, dh * 512:(dh + 1) * 512]
                            if hb == 0:
                                nc.vector.tensor_copy(dst, ps2[:])
                            else:
                                nc.vector.tensor_add(dst, dst, ps2[:])
                        if hb == HB - 1:
                            # scale + store as soon as a ts finishes
                            nc.vector.tensor_scalar_mul(
                                yas[ts][:], yas[ts][:],
                                wtt[:, ts0 + ts:ts0 + ts + 1])
                            q = nc.gpsimd if ts % 2 == 0 else nc.sync
                            q.dma_start(y_r[:, ts0 + ts, :], yas[ts][:])

                off += mega
                off16 += n1
                off8 += n3

    nc.compile()
    return nc


def _get_kernel(megas):
    megas = tuple(megas)
    if megas not in _KERNEL_CACHE:
        _KERNEL_CACHE[megas] = _build_kernel(megas)
    return _KERNEL_CACHE[megas]


def _route(xt, Wg, top_k):
    logits = xt.astype(np.float64) @ Wg.astype(np.float64)
    m = logits.max(axis=-1, keepdims=True)
    p = np.exp(logits - m)
    p /= p.sum(axis=-1, keepdims=True)
    order = np.argsort(-p, axis=-1, kind="stable")
    idx = order[:, :top_k]
    vals = np.take_along_axis(p, idx, axis=-1)
    w = vals / vals.sum(axis=-1, keepdims=True)
    return idx, w


def _pack(loads):
    """Pick uniform per-core mega sizes (SA, SB) and assign each expert
    exactly two slots (possibly on different cores).  Returns
    (SA, SB, k, assign): assign = [(expert, [("A"|"B", core), ...])]."""
    order = np.argsort(-loads, kind="stable")
    ls = loads[order]
    best = None
    for Ctot in range(2048, 4096 + 1, 128):
        for SA in range((Ctot + 255) // 256 * 128, Ctot - 511, 128):
            SB = Ctot - SA
            if SB < 512 or SB > SA:
                continue
            for k in range(0, 5):
                nmid = E - 2 * k
                if nmid < 0:
                    continue
                ok = (all(ls[i] <= 2 * SA for i in range(k))
                      and all(ls[i] <= SA + SB for i in range(k, k + nmid))
                      and all(ls[i] <= 2 * SB for i in range(k + nmid, E)))
                if ok:
                    best = (SA, SB, k)
                    break
            if best:
                break
        if best:
            break
    assert best is not None, f"no packing for loads {loads}"
    SA, SB, k = best
    slotsA = list(range(E))
    slotsB = list(range(E))
    assign = []
    ai = bi = 0
    for i in range(E):
        e = order[i]
        if i < k:
            s = [("A", slotsA[ai]), ("A", slotsA[ai + 1])]
            ai += 2
        elif i < k + (E - 2 * k):
            s = [("A", slotsA[ai]), ("B", slotsB[bi])]
            ai += 1
            bi += 1
        else:
            s = [("B", slotsB[bi]), ("B", slotsB[bi + 1])]
            bi += 2
        assign.append((e, s))
    return SA, SB, k, assign


def _expert_n3(cs, cap3, cap1):
    """fp8 token count for an expert with capacities (cap3, cap1)."""
    L = len(cs)
    n3 = max(L - cap1, 0)
    n3 = max(n3, int(np.searchsorted(cs, TH_FLOOR, side="right")))
    return min(n3, L)


def _choose_t3(SA, SB, assign, csorted):
    """Choose per-slot-type fp8 token counts (a3, b3): maximize fp8
    capacity subject to fit and the summed-c^2 error budget.
    csorted: {expert: ascending combine weights}.  Returns (a3, b3)."""
    best = (0, 0)
    best_key = (-1, 0.0)
    for a3 in range(0, SA - 127, 128):
        for b3 in range(0, SB - 127, 128):
            sumc2 = 0.0
            ok = True
            for e, slots in assign:
                cs = csorted[e]
                cap3 = sum(a3 if which == "A" else b3
                           for which, _ in slots)
                cap1 = sum((SA - a3) if which == "A" else (SB - b3)
                           for which, _ in slots)
                n3 = _expert_n3(cs, cap3, cap1)
                if n3 > cap3:
                    ok = False
                    break
                sumc2 += float(np.sum(cs[:n3] ** 2))
            if ok and sumc2 <= SUMC2_BUDGET:
                key = (a3 + b3, -sumc2)
                if key > best_key:
                    best_key = key
                    best = (a3, b3)
    return best


def kernel(x, Wg, W1, b1, W2, b2, top_k):
    import concourse.bass_utils as bass_utils

    top_k = int(top_k)
    B, S, d = x.shape
    T = B * S
    xt = np.ascontiguousarray(np.asarray(x, dtype=np.float32).reshape(T, d))
    Wg = np.asarray(Wg, dtype=np.float32)
    W1 = np.asarray(W1, dtype=np.float32)
    b1 = np.asarray(b1, dtype=np.float32)
    W2 = np.asarray(W2, dtype=np.float32)
    b2 = np.asarray(b2, dtype=np.float32)

    idx, w = _route(xt, Wg, top_k)

    # weight swizzles (fp16 baseline + scaled fp8)
    W1h = np.ascontiguousarray(
        W1.astype(np.float16)
        .reshape(E, KD, 128, HB, HBLK).transpose(0, 2, 3, 1, 4))
    W2h = np.ascontiguousarray(
        W2.astype(np.float16)
        .reshape(E, HB, KHB, 128, D).transpose(0, 3, 1, 2, 4))
    # fp8: [E, 128p, HB, KD2, KHB, 2, 128] from W1[e, d, h],
    # d = k2*256 + i*128 + p, h = hb*1024 + hs*128 + m (contiguous
    # (2, 128) pairs per (k2, hs) -> single-run fp8 LDWEIGHTS)
    W18 = np.ascontiguousarray(
        (W1 * W1_SCALE).astype(NPF8)
        .reshape(E, KD2, 2, 128, HB, KHB, 128)
        .transpose(0, 3, 4, 1, 5, 2, 6))
    # fp8: [E, 128p, HB, KD2, 2, D] from W2[e, h, d], h(within hb) =
    # k2*256+i*128+p
    W28 = np.ascontiguousarray(
        (W2 * W2_SCALE).astype(NPF8)
        .reshape(E, HB, KD2, 2, 128, D).transpose(0, 4, 1, 2, 3, 5))
    b1h = np.ascontiguousarray(
        b1.reshape(E, H // 128, 128).transpose(0, 2, 1))

    # per-expert token lists sorted by ascending combine weight
    toks = []
    wts_host = []
    csorted = {}
    for e in range(E):
        tlist = []
        for r in range(top_k):
            sel = np.nonzero(idx[:, r] == e)[0]
            for t in sel:
                tlist.append((w[t, r], t))
        tlist.sort()
        toks.append(np.array([t for _, t in tlist], dtype=np.int64))
        wts_host.append(np.array([c for c, _ in tlist], dtype=np.float32))
        csorted[e] = np.array([c for c, _ in tlist])
    loads = np.array([len(t) for t in toks])

    SA, SB, _k, assign = _pack(loads)
    a3, b3 = _choose_t3(SA, SB, assign, csorted)
    megas = ((SA, a3), (SB, b3))
    C = SA + SB
    C3 = a3 + b3
    nc = _get_kernel(megas)

    # token layout per mega: [fp8 region n3 (low-c first, pads at end)
    #                         | fp16 region]
    # xT16/xT8 are packed per-slice in DMA layout.
    x16 = xt.astype(np.float16)
    x8 = xt.astype(NPF8)

    xTe16 = [np.zeros((128, KD, C - C3), dtype=np.float16)
             for _ in range(N_CORES)]
    xTe8 = [np.zeros((128, KD2, 2, C3), dtype=NPF8) for _ in range(N_CORES)]
    wte = [np.zeros((C,), dtype=np.float32) for _ in range(N_CORES)]
    wmaps = [{} for _ in range(N_CORES)]
    scatter = []   # (core, mega_off, n, token_indices)

    # mega offsets: mega0 = A [0, SA): fp8 [0, a3) fp16 [a3, SA);
    # mega1 = B [SA, SA+SB): fp8 [SA, SA+b3) ...
    # xT8 index space: A-fp8 [0, a3), B-fp8 [a3, a3+b3)
    # xT16 index space: A-fp16 [0, SA-a3), B-fp16 [SA-a3, ...)
    reg = {
        "A": dict(moff=0, n3=a3, c_off8=0, c_off16=0),
        "B": dict(moff=SA, n3=b3, c_off8=a3, c_off16=SA - a3),
    }

    for e, slots in assign:
        cs = csorted[e]
        L = len(cs)
        cap3 = sum(a3 if which == "A" else b3 for which, _ in slots)
        cap1 = sum((SA - a3) if which == "A" else (SB - b3)
                   for which, _ in slots)
        n3 = min(_expert_n3(cs, cap3, cap1), cap3)
        # place fp8 tokens [0, n3), fp16 tokens [n3, L)
        pos3, pos1 = 0, n3
        for which, core in slots:
            r = reg[which]
            scap3 = a3 if which == "A" else b3
            scap1 = (SA - a3) if which == "A" else (SB - b3)
            # fp8 part
            n = min(scap3, n3 - pos3)
            if n > 0:
                tk = toks[e][pos3:pos3 + n]
                xTe8[core][:, :, :, r["c_off8"]:r["c_off8"] + n] = (
                    x8[tk].reshape(n, KD2, 2, 128).transpose(3, 1, 2, 0))
                wte[core][r["moff"]:r["moff"] + n] = (
                    wts_host[e][pos3:pos3 + n] / W2_SCALE)
                scatter.append((core, r["moff"], n, tk))
                pos3 += n
            # fp16 part
            m = min(scap1, L - pos1)
            if m > 0:
                tk = toks[e][pos1:pos1 + m]
                xTe16[core][:, :, r["c_off16"]:r["c_off16"] + m] = (
                    x16[tk].reshape(m, KD, 128).transpose(2, 1, 0))
                wte[core][r["moff"] + scap3:r["moff"] + scap3 + m] = (
                    wts_host[e][pos1:pos1 + m])
                scatter.append((core, r["moff"] + scap3, m, tk))
                pos1 += m
            mi = 0 if which == "A" else 1
            if scap1 > 0:
                wmaps[core][f"w1f16_{mi}"] = W1h[e]
                wmaps[core][f"w2f16_{mi}"] = W2h[e]
            if scap3 > 0:
                wmaps[core][f"w1f8_{mi}"] = W18[e]
                wmaps[core][f"w2f8_{mi}"] = W28[e]
            wmaps[core][f"b1_{mi}"] = b1h[e]
            if mi == 0:
                if a3 > 0:
                    wmaps[core]["w1h0a"] = np.ascontiguousarray(
                        W18[e][:, 0, :, :2]).reshape(128, -1)
                    wmaps[core]["w1h0b"] = np.ascontiguousarray(
                        W18[e][:, 0, :, 2:]).reshape(128, -1)
                else:
                    wmaps[core]["w1h0a"] = np.ascontiguousarray(
                        W1h[e][:, 0, :, :256]).reshape(128, -1)
                    wmaps[core]["w1h0b"] = np.ascontiguousarray(
                        W1h[e][:, 0, :, 256:]).reshape(128, -1)
        assert pos3 == n3 and pos1 == L, \
            f"expert {e} tokens not fully placed ({pos3}/{n3}, {pos1}/{L})"

    # flatten x into the per-slice DMA layouts
    spans16 = []
    spans8 = []
    for (sz, n3) in megas:
        base8 = spans8[-1][0] + spans8[-1][1] if spans8 else 0
        base16 = spans16[-1][0] + spans16[-1][1] if spans16 else 0
        for (soff, slen) in _best_slices(n3):
            spans8.append((base8 + soff, slen))
        for (soff, slen) in _best_slices(sz - n3):
            spans16.append((base16 + soff, slen))
    in_maps = []
    for c in range(N_CORES):
        m = {"wt": np.ascontiguousarray(wte[c].reshape(C // 128, 128).T)}
        if C > C3:
            xdev = np.empty((128, (C - C3) * KD), dtype=np.float16)
            for (a, slen) in spans16:
                xdev[:, a * KD:(a + slen) * KD] = (
                    xTe16[c][:, :, a:a + slen].reshape(128, -1))
            m["xT16"] = xdev
        if C3:
            xdev8 = np.empty((128, C3 * KD), dtype=NPF8)
            for (a, slen) in spans8:
                xdev8[:, a * KD:(a + slen) * KD] = (
                    xTe8[c][:, :, :, a:a + slen].reshape(128, -1))
            m["xT8"] = xdev8
        m.update(wmaps[c])
        in_maps.append(m)

    trace = os.environ.get("MOE_TRACE", "") not in ("", "0")
    run_kwargs = {}
    if trace:
        _install_ntff_hook()
        run_kwargs = dict(
            trace=True,
            trace_cores=[int(c) for c in
                         os.environ.get("MOE_TRACE_CORES", "0").split(",")],
            tmpdir=os.environ.get("MOE_TRACE_DIR") or None,
        )
    res = bass_utils.run_bass_kernel_spmd(
        nc, in_maps, core_ids=list(range(N_CORES)), **run_kwargs)
    if trace:
        global LAST_EXEC_NS
        LAST_EXEC_NS = res.exec_time_ns
        print(f"MOE exec_time_ns: {res.exec_time_ns}")
        if res.instructions_and_trace:
            print(f"MOE trace: {res.instructions_and_trace[1]}")

    out = np.zeros((T, D), dtype=np.float32)
    for core, moff, n, tk in scatter:
        out[tk] += res.results[core]["y"][moff:moff + n]
    combine = np.zeros((T, E), dtype=np.float32)
    np.put_along_axis(combine, idx, w.astype(np.float32), axis=1)
    out += combine @ b2

    return out.reshape(B, S, d).astype(np.float32)


def _install_ntff_hook():
    import sys, types
    if "antenv.axon_hooks" in sys.modules:
        return
    mod = types.ModuleType("antenv.axon_hooks")
    store = {"h": None}
    mod.set_axon_ntff_profile_hook = lambda h: store.__setitem__("h", h)
    mod.get_axon_ntff_profile_hook = lambda: store["h"]
    import antenv
    sys.modules["antenv.axon_hooks"] = mod
    antenv.axon_hooks = mod
    try:
        from trn_agent_boot.trn_boot import _ntff_profile_via_ctypes
        mod.set_axon_ntff_profile_hook(
            _ntff_profile_via_ctypes("/opt/axon/libaxon_pjrt.so"))
    except Exception as exc:
        print(f"ntff hook install failed: {exc}")


# revision 25
# speedup vs baseline: 1.0052x; 1.0052x over previous
"""MoE (top-k of 8 experts) Trainium2 kernel — mixed fp16/fp8 tiers.

Strategy (expert parallelism + per-assignment precision tiering):
  - Host computes gating (float64 softmax/top-k/renorm) exactly as the
    reference.
  - Each (token, expert) assignment runs either the fp16 path (512 PE
    cyc/token) or, when its combine weight c is small, the full-fp8
    path (256 cyc/token): fp8 e4m3 matmuls in DoubleRow perf mode
    process K=256 per instruction at the fp16 row rate (2x FLOPs).
    Measured e4m3 pipeline error ~7.7% x c per fp8 assignment; tokens
    are tiered so total rel err stays ~1.6e-2 (< 2e-2 gate).
  - Weights pre-scaled before fp8 quantization (W1 x32, W2 x64) to
    escape e4m3's subnormal range; dequant is folded into the ACT
    scale (GEMM1) and the host-packed per-token combine weights
    (GEMM2).
  - Packing: per-core 2 mega-slots (SA, SB) as in the fp16 baseline,
    each mega = one expert's tokens with a leading tier3 (fp8) block
    region (a3 / b3 tokens) and a fp16 tail; per-expert tier3 counts
    are capacity-driven with an error-threshold feasibility check.
  - Host scatter-adds expert contributions + combine-weighted b2.

Device kernel (per core, per mega, per 1024-row weight block hb):
  GEMM1 fp8 : psum[h,t] = sum_k2 (32*W1)_8[.,k2,2,h].T @ x8[.,k2,2,t]
              (DoubleRow), ACT: ht8 = fp8(gelu(psum/32 + b1)).
  GEMM1 fp16: baseline path -> ht16 = fp16(gelu(psum + b1)).
  GEMM2 fp8 : ps2[t,d] += ht8[.,k2,2,t].T @ (64*W2)_8[.,k2,2,d]
              (DoubleRow, 4 k-steps), DVE-accumulated into yas.
  GEMM2 fp16: baseline path (8 k-steps).
  yas scaled by host wt' (wt/64 for fp8 blocks) and stored per ts.
"""

import os
import numpy as np
import ml_dtypes

D = 1024
H = 4096
E = 8
N_CORES = 8
HBLK = 1024          # h rows per streamed weight block
HB = H // HBLK       # 4 blocks
KD = D // 128        # 8 k128-tiles for GEMM1 (fp16)
KD2 = KD // 2        # 4 k256-tiles for GEMM1 (fp8 DoubleRow)
KHB = HBLK // 128    # 8 k128-tiles per block for GEMM2 (fp16)
KHB2 = KHB // 2      # 4 k256-tiles for GEMM2 (fp8)

W1_SCALE = 32.0
W2_SCALE = 64.0
TH_FLOOR = 0.37      # fp8 if c <= floor (when capacity allows)
# Predicted rel err of fp8 tiering ~= ERR_K * sqrt(sum of c^2 over fp8
# assignments); ERR_K calibrated by exact host sim of the e4m3 pipeline
# on reference-scale inputs.  Budget keeps predicted rel <= ~1.7e-2.
ERR_K = 8.09e-4
SUMC2_BUDGET = 452.0

NPF8 = ml_dtypes.float8_e4m3


def _slice_period(n):
    # fp16 matmul issue period (measured): N/2.4GHz + ~3ns dispatch,
    # with a ~100ns floor where the FWL LDWEIGHTS (~97ns) stops being
    # hidden by the moving-operand stream.
    return max(n / 2.4 + 3.0, 100.0)


def _best_slices(mega):
    """DP: split mega into moving-dim slices (multiples of 64, <=512)
    minimizing the summed matmul issue period."""
    if mega == 0:
        return []
    best = {0: (0.0, ())}
    for m in range(64, mega + 64, 64):
        cands = []
        for s in range(64, min(512, m) + 64, 64):
            if m - s in best:
                c, parts = best[m - s]
                cands.append((c + _slice_period(s), parts + (s,)))
        if cands:
            best[m] = min(cands)
    assert mega in best, f"no slice decomposition for {mega}"
    _, parts = best[mega]
    out = []
    off = 0
    for s in parts:
        out.append((off, s))
        off += s
    return out


_KERNEL_CACHE = {}
LAST_EXEC_NS = None


def _build_kernel(megas):
    """megas: tuple of (size, n3) per mega; n3 = leading fp8 tokens
    (multiple of 128), size multiple of 128."""
    import concourse.bacc as bacc
    import concourse.mybir as mybir
    import concourse.tile as tile

    f32 = mybir.dt.float32
    f16 = mybir.dt.float16
    f8 = mybir.dt.float8e4
    GELU = mybir.ActivationFunctionType.Gelu_apprx_tanh
    DR = mybir.MatmulPerfMode.DoubleRow

    C = sum(sz for sz, _ in megas)
    C3 = sum(n3 for _, n3 in megas)
    nc = bacc.Bacc("TRN2", target_bir_lowering=False, debug=False,
                   num_devices=N_CORES)

    # host-swizzled layouts matching SBUF tile layouts (128-row DMAs of
    # long contiguous runs):
    #   xT16[p, per-slice (kk, c)]   xT8[p, per-slice (k2, 2, c)]
    #   w1f16[p, hb, kk, hw]         w1f8[p, hb, k2, hs, 2, 128]
    #   w2f16[p, hb, kh, d]          w2f8[p, hb, k2, 2, d]
    xT16 = (nc.dram_tensor("xT16", [128, (C - C3) * KD], f16,
                           kind="ExternalInput").ap()
            if C > C3 else None)
    xT8 = (nc.dram_tensor("xT8", [128, C3 * KD], f8,
                          kind="ExternalInput").ap()
           if C3 else None)
    wts = []
    for mi, (sz, n3) in enumerate(megas):
        ww = {}
        if sz > n3:
            ww["w1f16"] = nc.dram_tensor(f"w1f16_{mi}", [128, HB, KD, HBLK],
                                         f16, kind="ExternalInput").ap()
            ww["w2f16"] = nc.dram_tensor(f"w2f16_{mi}", [128, HB, KHB, D],
                                         f16, kind="ExternalInput").ap()
        if n3:
            ww["w1f8"] = nc.dram_tensor(f"w1f8_{mi}",
                                        [128, HB, KD2, KHB, 2, 128],
                                        f8, kind="ExternalInput").ap()
            ww["w2f8"] = nc.dram_tensor(f"w2f8_{mi}", [128, HB, KD2, 2, D],
                                        f8, kind="ExternalInput").ap()
        # pre-transposed on host: [128, H/128], col j = b1[j*128 + p]
        ww["b1"] = nc.dram_tensor(f"b1_{mi}", [128, H // 128], f32,
                                  kind="ExternalInput").ap()
        wts.append(ww)
    # pre-transposed + tier-scaled on host: [128, C/128]
    wt = nc.dram_tensor("wt", [128, C // 128], f32,
                        kind="ExternalInput").ap()
    # mega0 W1 block-0 prestage: fp8 part as two fast-issue chunks (the
    # first fp8 matmuls wait only on the small "a" chunk); if mega0 has
    # no fp8 region, prestage the fp16 block instead.
    m0_has8 = megas[0][1] > 0
    if m0_has8:
        w1h0a = nc.dram_tensor("w1h0a", [128, KD2 * 2 * 2 * 128], f8,
                               kind="ExternalInput").ap()
        w1h0b = nc.dram_tensor("w1h0b", [128, KD2 * 6 * 2 * 128], f8,
                               kind="ExternalInput").ap()
    else:
        w1h0a = nc.dram_tensor("w1h0a", [128, KD * 256], f16,
                               kind="ExternalInput").ap()
        w1h0b = nc.dram_tensor("w1h0b", [128, KD * 768], f16,
                               kind="ExternalInput").ap()
    y = nc.dram_tensor("y", [C, D], f32, kind="ExternalOutput").ap()

    with tile.TileContext(nc) as tc:
        with (
            tc.tile_pool(name="meta", bufs=1) as pmeta,
            tc.tile_pool(name="xg", bufs=3) as pxg,
            tc.tile_pool(name="yacc", bufs=10) as pyacc,
            tc.tile_pool(name="w1p", bufs=2) as pw1,
            tc.tile_pool(name="w1p8", bufs=2) as pw18,
            tc.tile_pool(name="w2p", bufs=1) as pw2,
            tc.tile_pool(name="w2p8", bufs=1) as pw28,
            tc.tile_pool(name="hact", bufs=1) as phact,
            tc.tile_pool(name="ps1", bufs=4, space="PSUM") as pps1,
            tc.tile_pool(name="ps2", bufs=4, space="PSUM") as pps2,
        ):
            y_r = y.rearrange("(t p) d -> p t d", p=128)
            wtt = None

            # PE warmup on zeros during the DMA head (pstate ramp).
            warm = pmeta.tile([128, 512], f16, name="warm")
            nc.vector.memset(warm[:], 0.0)
            # preload the gelu ACT table (lazy-loads on first ACTIVATE,
            # ~1.3us) off the critical path
            warmact = pmeta.tile([128, 1], f32, name="warmact")
            nc.scalar.activation(warmact[:], warm[:, :1], GELU)
            for wi in range(14):
                pw = pps1.tile([128, 512], f32, tag="ps1",
                               name=f"warm_ps_{wi}")
                nc.tensor.matmul(pw[:], warm[:, :128], warm[:],
                                 start=True, stop=True)

            if m0_has8:
                w1h0a_t = pmeta.tile([128, KD2, 2, 2, 128], f8,
                                     name="w1h0a_t")
                nc.sync.dma_start(
                    w1h0a_t[:],
                    w1h0a.rearrange("p (k s i m) -> p k s i m",
                                    k=KD2, s=2, i=2))
                w1h0b_t = pmeta.tile([128, KD2, 6, 2, 128], f8,
                                     name="w1h0b_t")
                nc.sync.dma_start(
                    w1h0b_t[:],
                    w1h0b.rearrange("p (k s i m) -> p k s i m",
                                    k=KD2, s=6, i=2))
            else:
                w1h0a_t = pmeta.tile([128, KD, 256], f16, name="w1h0a_t")
                nc.sync.dma_start(
                    w1h0a_t[:], w1h0a.rearrange("p (kk h) -> p kk h", kk=KD))
                w1h0b_t = pmeta.tile([128, KD, 768], f16, name="w1h0b_t")
                nc.sync.dma_start(
                    w1h0b_t[:], w1h0b.rearrange("p (kk h) -> p kk h", kk=KD))

            off = 0        # token offset into C
            off16 = 0      # fp16 token offset (into xT16)
            off8 = 0       # fp8 token offset (into xT8)
            for mi, (mega, n3) in enumerate(megas):
                ww = wts[mi]
                n1 = mega - n3
                ts_count = mega // 128
                nt3 = n3 // 128          # fp8 ts-blocks (leading)
                ts0 = off // 128
                sl3 = _best_slices(n3)
                sl1 = _best_slices(n1)

                # per-slice token loads (gpsimd queue so they don't
                # serialize behind the weight streams)
                xg8s = []
                for (soff, slen) in sl3:
                    xg = pxg.tile([128, KD2, 2, slen], f8, tag="xg8",
                                  name=f"xg8_{mi}_{soff}")
                    base = (off8 + soff) * KD
                    nc.gpsimd.dma_start(
                        xg[:],
                        xT8[:, base:base + slen * KD]
                        .rearrange("p (k i c) -> p k i c", k=KD2, i=2))
                    xg8s.append(xg)
                xg16s = []
                for (soff, slen) in sl1:
                    xg = pxg.tile([128, KD, slen], f16, tag="xg16",
                                  name=f"xg16_{mi}_{soff}")
                    base = (off16 + soff) * KD
                    # scalar queue: keeps the head's gpsimd/sync queues
                    # free for the critical fp8 prestage + x8 loads
                    nc.scalar.dma_start(
                        xg[:],
                        xT16[:, base:base + slen * KD]
                        .rearrange("p (kk c) -> p kk c", kk=KD))
                    xg16s.append(xg)

                b1t = pmeta.tile([128, H // 128], f32, tag=f"b1_{mi}")
                nc.sync.dma_start(b1t[:], ww["b1"][:])
                if wtt is None:
                    wtt = pmeta.tile([128, C // 128], f32, name="wtt")
                    nc.sync.dma_start(wtt[:], wt[:])

                yas = [pyacc.tile([128, D], f32, tag="ya",
                                  name=f"ya_{mi}_{ts}")
                       for ts in range(ts_count)]

                for hb in range(HB):
                    first_blk = mi == 0 and hb == 0
                    # --- weight block loads (sync queue); issue order =
                    # consumption order after the per-hb reordering:
                    # w1f8, w2f8, w1f16, w2f16 ---
                    if n3:
                        if first_blk and m0_has8:
                            w1t8 = None     # served from w1h0a/b tiles
                        else:
                            w1t8 = pw18.tile([128, KD2, KHB, 2, 128], f8,
                                             tag="w1t8")
                            nc.sync.dma_start(w1t8[:], ww["w1f8"][:, hb])
                        w2t8 = pw28.tile([128, KD2, 2, D], f8, tag="w2t8")
                        nc.sync.dma_start(w2t8[:], ww["w2f8"][:, hb])
                    if n1:
                        if first_blk and not m0_has8:
                            w1t = None      # served from w1h0a/b tiles
                        else:
                            w1t = pw1.tile([128, KD, HBLK], f16, tag="w1t")
                            nc.sync.dma_start(w1t[:], ww["w1f16"][:, hb])
                        w2t = pw2.tile([128, KHB, D], f16, tag="w2t")
                        nc.sync.dma_start(w2t[:], ww["w2f16"][:, hb])

                    if n3:
                        ht8 = phact.tile([128, KHB2, 2, n3], f8, tag="ht8")
                    if n1:
                        ht = phact.tile([128, KHB, n1], f16, tag="ht")

                    # --- GEMM1 fp8 (DoubleRow) + gelu -> ht8 ---
                    for si, (soff, slen) in enumerate(sl3):
                        for hs in range(KHB):
                            ps = pps1.tile([128, 512], f32, tag="ps1")
                            for k in range(KD2):
                                if first_blk and m0_has8:
                                    w1s = (w1h0a_t[:, k, hs, :, :]
                                           if hs < 2 else
                                           w1h0b_t[:, k, hs - 2, :, :])
                                else:
                                    w1s = w1t8[:, k, hs, :, :]
                                nc.tensor.matmul(
                                    ps[:, :slen],
                                    w1s,
                                    xg8s[si][:, k, :, :],
                                    start=(k == 0), stop=(k == KD2 - 1),
                                    perf_mode=DR,
                                )
                            nc.scalar.activation(
                                ht8[:, hs // 2, hs % 2, soff:soff + slen],
                                ps[:, :slen], GELU,
                                bias=b1t[:, hb * KHB + hs:hb * KHB + hs + 1],
                                scale=1.0 / W1_SCALE,
                            )

                    # --- GEMM2 fp8 (before the fp16 sections: hides the
                    # fp16 W1 stream latency behind fp8 work) ---
                    for ts in range(nt3):
                        for dh in range(2):
                            ps2 = pps2.tile([128, 512], f32, tag="ps2")
                            for k in range(KHB2):
                                nc.tensor.matmul(
                                    ps2[:],
                                    ht8[:, k, :, ts * 128:(ts + 1) * 128],
                                    w2t8[:, k, :, dh * 512:(dh + 1) * 512],
                                    start=(k == 0), stop=(k == KHB2 - 1),
                                    perf_mode=DR,
                                )
                            dst = yas[ts][:, dh * 512:(dh + 1) * 512]
                            if hb == 0:
                                nc.vector.tensor_copy(dst, ps2[:])
                            else:
                                nc.vector.tensor_add(dst, dst, ps2[:])
                        if hb == HB - 1:
                            nc.vector.tensor_scalar_mul(
                                yas[ts][:], yas[ts][:],
                                wtt[:, ts0 + ts:ts0 + ts + 1])
                            q = nc.gpsimd if ts % 2 == 0 else nc.sync
                            q.dma_start(y_r[:, ts0 + ts, :], yas[ts][:])

                    # --- GEMM1 fp16 + gelu -> ht ---
                    if first_blk and not m0_has8:
                        ns = len(sl1)
                        order = ([(si, hs) for si in range(min(2, ns))
                                  for hs in range(2)]
                                 + [(si, hs) for si in range(min(2, ns))
                                    for hs in range(2, KHB)]
                                 + [(si, hs) for si in range(2, ns)
                                    for hs in range(KHB)])
                    else:
                        order = [(si, hs) for si in range(len(sl1))
                                 for hs in range(KHB)]
                    for si, hs in order:
                        soff, slen = sl1[si]
                        ps = pps1.tile([128, 512], f32, tag="ps1")
                        for k in range(KD):
                            if first_blk and not m0_has8:
                                w1s = (
                                    w1h0a_t[:, k, hs * 128:(hs + 1) * 128]
                                    if hs < 2 else
                                    w1h0b_t[:, k,
                                            (hs - 2) * 128:(hs - 1) * 128])
                            else:
                                w1s = w1t[:, k, hs * 128:(hs + 1) * 128]
                            nc.tensor.matmul(
                                ps[:, :slen],
                                w1s,
                                xg16s[si][:, k, :],
                                start=(k == 0), stop=(k == KD - 1),
                            )
                        nc.scalar.activation(
                            ht[:, hs, soff:soff + slen], ps[:, :slen],
                            GELU,
                            bias=b1t[:, hb * KHB + hs:hb * KHB + hs + 1],
                        )

                    # --- GEMM2 fp16 ---
                    for ts in range(nt3, ts_count):
                        tl = ts * 128 - n3
                        for dh in range(2):
                            ps2 = pps2.tile([128, 512], f32, tag="ps2")
                            for k in range(KHB):
                                nc.tensor.matmul(
                                    ps2[:],
                                    ht[:, k, tl:tl + 128],
                                    w2t[:, k, dh * 512:(dh + 1) * 512],
                                    start=(k == 0), stop=(k == KHB - 1),
                                )
                            dst = yas[ts][:, dh * 512:(dh + 1) * 512]
                            if hb == 0:
                                nc.vector.tensor_copy(dst, ps2[:])
                            else:
                                nc.vector.tensor_add(dst, dst, ps2[:])
                        if hb == HB - 1:
                            # scale + store as soon as a ts finishes
                            nc.vector.tensor_scalar_mul(
                                yas[ts][:], yas[ts][:],
                                wtt[:, ts0 + ts:ts0 + ts + 1])
                            q = nc.gpsimd if ts % 2 == 0 else nc.sync
                            q.dma_start(y_r[:, ts0 + ts, :], yas[ts][:])

                off += mega
                off16 += n1
                off8 += n3

    nc.compile()
    return nc


def _get_kernel(megas):
    megas = tuple(megas)
    if megas not in _KERNEL_CACHE:
        _KERNEL_CACHE[megas] = _build_kernel(megas)
    return _KERNEL_CACHE[megas]


def _route(xt, Wg, top_k):
    logits = xt.astype(np.float64) @ Wg.astype(np.float64)
    m = logits.max(axis=-1, keepdims=True)
    p = np.exp(logits - m)
    p /= p.sum(axis=-1, keepdims=True)
    order = np.argsort(-p, axis=-1, kind="stable")
    idx = order[:, :top_k]
    vals = np.take_along_axis(p, idx, axis=-1)
    w = vals / vals.sum(axis=-1, keepdims=True)
    return idx, w


def _pack(loads):
    """Pick uniform per-core mega sizes (SA, SB) and assign each expert
    exactly two slots (possibly on different cores).  Returns
    (SA, SB, k, assign): assign = [(expert, [("A"|"B", core), ...])]."""
    order = np.argsort(-loads, kind="stable")
    ls = loads[order]
    best = None
    for Ctot in range(2048, 4096 + 1, 128):
        for SA in range((Ctot + 255) // 256 * 128, Ctot - 511, 128):
            SB = Ctot - SA
            if SB < 512 or SB > SA:
                continue
            for k in range(0, 5):
                nmid = E - 2 * k
                if nmid < 0:
                    continue
                ok = (all(ls[i] <= 2 * SA for i in range(k))
                      and all(ls[i] <= SA + SB for i in range(k, k + nmid))
                      and all(ls[i] <= 2 * SB for i in range(k + nmid, E)))
                if ok:
                    best = (SA, SB, k)
                    break
            if best:
                break
        if best:
            break
    assert best is not None, f"no packing for loads {loads}"
    SA, SB, k = best
    slotsA = list(range(E))
    slotsB = list(range(E))
    assign = []
    ai = bi = 0
    for i in range(E):
        e = order[i]
        if i < k:
            s = [("A", slotsA[ai]), ("A", slotsA[ai + 1])]
            ai += 2
        elif i < k + (E - 2 * k):
            s = [("A", slotsA[ai]), ("B", slotsB[bi])]
            ai += 1
            bi += 1
        else:
            s = [("B", slotsB[bi]), ("B", slotsB[bi + 1])]
            bi += 2
        assign.append((e, s))
    return SA, SB, k, assign


def _expert_n3(cs, cap3, cap1):
    """fp8 token count for an expert with capacities (cap3, cap1)."""
    L = len(cs)
    n3 = max(L - cap1, 0)
    n3 = max(n3, int(np.searchsorted(cs, TH_FLOOR, side="right")))
    return min(n3, L)


def _choose_t3(SA, SB, assign, csorted):
    """Choose per-slot-type fp8 token counts (a3, b3): maximize fp8
    capacity subject to fit and the summed-c^2 error budget.
    csorted: {expert: ascending combine weights}.  Returns (a3, b3)."""
    best = (0, 0)
    best_key = (-1, 0.0)
    for a3 in range(0, SA - 127, 128):
        for b3 in range(0, SB - 127, 128):
            sumc2 = 0.0
            ok = True
            for e, slots in assign:
                cs = csorted[e]
                cap3 = sum(a3 if which == "A" else b3
                           for which, _ in slots)
                cap1 = sum((SA - a3) if which == "A" else (SB - b3)
                           for which, _ in slots)
                n3 = _expert_n3(cs, cap3, cap1)
                if n3 > cap3:
                    ok = False
                    break
                sumc2 += float(np.sum(cs[:n3] ** 2))
            if ok and sumc2 <= SUMC2_BUDGET:
                key = (a3 + b3, -sumc2)
                if key > best_key:
                    best_key = key
                    best = (a3, b3)
    return best


def kernel(x, Wg, W1, b1, W2, b2, top_k):
    import concourse.bass_utils as bass_utils

    top_k = int(top_k)
    B, S, d = x.shape
    T = B * S
    xt = np.ascontiguousarray(np.asarray(x, dtype=np.float32).reshape(T, d))
    Wg = np.asarray(Wg, dtype=np.float32)
    W1 = np.asarray(W1, dtype=np.float32)
    b1 = np.asarray(b1, dtype=np.float32)
    W2 = np.asarray(W2, dtype=np.float32)
    b2 = np.asarray(b2, dtype=np.float32)

    idx, w = _route(xt, Wg, top_k)

    # weight swizzles (fp16 baseline + scaled fp8)
    W1h = np.ascontiguousarray(
        W1.astype(np.float16)
        .reshape(E, KD, 128, HB, HBLK).transpose(0, 2, 3, 1, 4))
    W2h = np.ascontiguousarray(
        W2.astype(np.float16)
        .reshape(E, HB, KHB, 128, D).transpose(0, 3, 1, 2, 4))
    # fp8: [E, 128p, HB, KD2, KHB, 2, 128] from W1[e, d, h],
    # d = k2*256 + i*128 + p, h = hb*1024 + hs*128 + m (contiguous
    # (2, 128) pairs per (k2, hs) -> single-run fp8 LDWEIGHTS)
    W18 = np.ascontiguousarray(
        (W1 * W1_SCALE).astype(NPF8)
        .reshape(E, KD2, 2, 128, HB, KHB, 128)
        .transpose(0, 3, 4, 1, 5, 2, 6))
    # fp8: [E, 128p, HB, KD2, 2, D] from W2[e, h, d], h(within hb) =
    # k2*256+i*128+p
    W28 = np.ascontiguousarray(
        (W2 * W2_SCALE).astype(NPF8)
        .reshape(E, HB, KD2, 2, 128, D).transpose(0, 4, 1, 2, 3, 5))
    b1h = np.ascontiguousarray(
        b1.reshape(E, H // 128, 128).transpose(0, 2, 1))

    # per-expert token lists sorted by ascending combine weight
    toks = []
    wts_host = []
    csorted = {}
    for e in range(E):
        tlist = []
        for r in range(top_k):
            sel = np.nonzero(idx[:, r] == e)[0]
            for t in sel:
                tlist.append((w[t, r], t))
        tlist.sort()
        toks.append(np.array([t for _, t in tlist], dtype=np.int64))
        wts_host.append(np.array([c for c, _ in tlist], dtype=np.float32))
        csorted[e] = np.array([c for c, _ in tlist])
    loads = np.array([len(t) for t in toks])

    SA, SB, _k, assign = _pack(loads)
    a3, b3 = _choose_t3(SA, SB, assign, csorted)
    megas = ((SA, a3), (SB, b3))
    C = SA + SB
    C3 = a3 + b3
    nc = _get_kernel(megas)

    # token layout per mega: [fp8 region n3 (low-c first, pads at end)
    #                         | fp16 region]
    # xT16/xT8 are packed per-slice in DMA layout.
    x16 = xt.astype(np.float16)
    x8 = xt.astype(NPF8)

    xTe16 = [np.zeros((128, KD, C - C3), dtype=np.float16)
             for _ in range(N_CORES)]
    xTe8 = [np.zeros((128, KD2, 2, C3), dtype=NPF8) for _ in range(N_CORES)]
    wte = [np.zeros((C,), dtype=np.float32) for _ in range(N_CORES)]
    wmaps = [{} for _ in range(N_CORES)]
    scatter = []   # (core, mega_off, n, token_indices)

    # mega offsets: mega0 = A [0, SA): fp8 [0, a3) fp16 [a3, SA);
    # mega1 = B [SA, SA+SB): fp8 [SA, SA+b3) ...
    # xT8 index space: A-fp8 [0, a3), B-fp8 [a3, a3+b3)
    # xT16 index space: A-fp16 [0, SA-a3), B-fp16 [SA-a3, ...)
    reg = {
        "A": dict(moff=0, n3=a3, c_off8=0, c_off16=0),
        "B": dict(moff=SA, n3=b3, c_off8=a3, c_off16=SA - a3),
    }

    for e, slots in assign:
        cs = csorted[e]
        L = len(cs)
        cap3 = sum(a3 if which == "A" else b3 for which, _ in slots)
        cap1 = sum((SA - a3) if which == "A" else (SB - b3)
                   for which, _ in slots)
        n3 = min(_expert_n3(cs, cap3, cap1), cap3)
        # place fp8 tokens [0, n3), fp16 tokens [n3, L)
        pos3, pos1 = 0, n3
        for which, core in slots:
            r = reg[which]
            scap3 = a3 if which == "A" else b3
            scap1 = (SA - a3) if which == "A" else (SB - b3)
            # fp8 part
            n = min(scap3, n3 - pos3)
            if n > 0:
                tk = toks[e][pos3:pos3 + n]
                xTe8[core][:, :, :, r["c_off8"]:r["c_off8"] + n] = (
                    x8[tk].reshape(n, KD2, 2, 128).transpose(3, 1, 2, 0))
                wte[core][r["moff"]:r["moff"] + n] = (
                    wts_host[e][pos3:pos3 + n] / W2_SCALE)
                scatter.append((core, r["moff"], n, tk))
                pos3 += n
            # fp16 part
            m = min(scap1, L - pos1)
            if m > 0:
                tk = toks[e][pos1:pos1 + m]
                xTe16[core][:, :, r["c_off16"]:r["c_off16"] + m] = (
                    x16[tk].reshape(m, KD, 128).transpose(2, 1, 0))
                wte[core][r["moff"] + scap3:r["moff"] + scap3 + m] = (
                    wts_host[e][pos1:pos1 + m])
                scatter.append((core, r["moff"] + scap3, m, tk))
                pos1 += m
            mi = 0 if which == "A" else 1
            if scap1 > 0:
                wmaps[core][f"w1f16_{mi}"] = W1h[e]
                wmaps[core][f"w2f16_{mi}"] = W2h[e]
            if scap3 > 0:
                wmaps[core][f"w1f8_{mi}"] = W18[e]
                wmaps[core][f"w2f8_{mi}"] = W28[e]
            wmaps[core][f"b1_{mi}"] = b1h[e]
            if mi == 0:
                if a3 > 0:
                    wmaps[core]["w1h0a"] = np.ascontiguousarray(
                        W18[e][:, 0, :, :2]).reshape(128, -1)
                    wmaps[core]["w1h0b"] = np.ascontiguousarray(
                        W18[e][:, 0, :, 2:]).reshape(128, -1)
                else:
                    wmaps[core]["w1h0a"] = np.ascontiguousarray(
                        W1h[e][:, 0, :, :256]).reshape(128, -1)
                    wmaps[core]["w1h0b"] = np.ascontiguousarray(
                        W1h[e][:, 0, :, 256:]).reshape(128, -1)
        assert pos3 == n3 and pos1 == L, \
            f"expert {e} tokens not fully placed ({pos3}/{n3}, {pos1}/{L})"

    # flatten x into the per-slice DMA layouts
    spans16 = []
    spans8 = []
    for (sz, n3) in megas:
        base8 = spans8[-1][0] + spans8[-1][1] if spans8 else 0
        base16 = spans16[-1][0] + spans16[-1][1] if spans16 else 0
        for (soff, slen) in _best_slices(n3):
            spans8.append((base8 + soff, slen))
        for (soff, slen) in _best_slices(sz - n3):
            spans16.append((base16 + soff, slen))
    in_maps = []
    for c in range(N_CORES):
        m = {"wt": np.ascontiguousarray(wte[c].reshape(C // 128, 128).T)}
        if C > C3:
            xdev = np.empty((128, (C - C3) * KD), dtype=np.float16)
            for (a, slen) in spans16:
                xdev[:, a * KD:(a + slen) * KD] = (
                    xTe16[c][:, :, a:a + slen].reshape(128, -1))
            m["xT16"] = xdev
        if C3:
            xdev8 = np.empty((128, C3 * KD), dtype=NPF8)
            for (a, slen) in spans8:
                xdev8[:, a * KD:(a + slen) * KD] = (
                    xTe8[c][:, :, :, a:a + slen].reshape(128, -1))
            m["xT8"] = xdev8
        m.update(wmaps[c])
        in_maps.append(m)

    trace = os.environ.get("MOE_TRACE", "") not in ("", "0")
    run_kwargs = {}
    if trace:
        _install_ntff_hook()
        run_kwargs = dict(
            trace=True,
            trace_cores=[int(c) for c in
                         os.environ.get("MOE_TRACE_CORES", "0").split(",")],
            tmpdir=os.environ.get("MOE_TRACE_DIR") or None,
        )
    res = bass_utils.run_bass_kernel_spmd(
        nc, in_maps, core_ids=list(range(N_CORES)), **run_kwargs)
    if trace:
        global LAST_EXEC_NS
        LAST_EXEC_NS = res.exec_time_ns
        print(f"MOE exec_time_ns: {res.exec_time_ns}")
        if res.instructions_and_trace:
            print(f"MOE trace: {res.instructions_and_trace[1]}")

    out = np.zeros((T, D), dtype=np.float32)
    for core, moff, n, tk in scatter:
        out[tk] += res.results[core]["y"][moff:moff + n]
    combine = np.zeros((T, E), dtype=np.float32)
    np.put_along_axis(combine, idx, w.astype(np.float32), axis=1)
    out += combine @ b2

    return out.reshape(B, S, d).astype(np.float32)


def _install_ntff_hook():
    import sys, types
    if "antenv.axon_hooks" in sys.modules:
        return
    mod = types.ModuleType("antenv.axon_hooks")
    store = {"h": None}
    mod.set_axon_ntff_profile_hook = lambda h: store.__setitem__("h", h)
    mod.get_axon_ntff_profile_hook = lambda: store["h"]
    import antenv
    sys.modules["antenv.axon_hooks"] = mod
    antenv.axon_hooks = mod
    try:
        from trn_agent_boot.trn_boot import _ntff_profile_via_ctypes
        mod.set_axon_ntff_profile_hook(
            _ntff_profile_via_ctypes("/opt/axon/libaxon_pjrt.so"))
    except Exception as exc:
        print(f"ntff hook install failed: {exc}")
